# revision 1
# baseline (speedup 1.0000x reference)
"""Mixed causal attention (B=8,L=1024,D=1024,H=16,NS=8) on 8 TRN2 cores.

Sharding: data-parallel over batch (core b owns batch b).  The per-position
(ns) projection weights are sharded by position: core c computes q/k/v for
position 1016+c for ALL batches, an AllGather shares the results, and each
core extracts its batch's 8 tail rows with a one-hot selection matmul.

v2 restructure vs baseline:
  - ns phase + AllGather run FIRST with dedicated DMA queue priority and
    PSUM accumulation (collective completes ~50us in, not ~180us).
  - input DMAs spread across vector/scalar/gpsimd queues.
  - DVE reciprocal (3.3us each) replaced by reciprocal_approx_fast.
  - key-padding folded into V (so exp needs no per-block bias) allowing
    fused exp over the fully-valid g1 key blocks.
  - score/PV matmuls narrowed to the causally-valid column range.
  - projection rounds interleaved with attention head-pair rounds so the
    tensor engine never idles (HAM stays warm).
"""

import sys
import os
from contextlib import ExitStack

import numpy as np

sys.path.insert(0, "/opt/trn_rl_repo")

import ml_dtypes  # noqa: E402
import concourse.bass as bass  # noqa: E402
import concourse.tile as tile  # noqa: E402
from concourse import bacc, mybir  # noqa: E402
from concourse._compat import with_exitstack  # noqa: E402
from concourse.bass_utils import run_bass_kernel_spmd  # noqa: E402

B, L, D, H, NS = 8, 1024, 1024, 16, 8
HD = D // H          # 64
LS = L - NS          # 1016
NCORES = 8
NEG = -1.0e9
BF = mybir.dt.bfloat16
F32 = mybir.dt.float32
EXP = mybir.ActivationFunctionType.Exp
ADD = mybir.AluOpType.add
MULT = mybir.AluOpType.mult

_CACHE = {}
TRACE = False
DEBUG = False


def _lead(g, j):
    return max(0, j * 128 - g * 512)


@with_exitstack
def _attention_kernel(ctx: ExitStack, tc: tile.TileContext, aps: dict):
    nc = tc.nc

    sb = ctx.enter_context(tc.tile_pool(name="persist", bufs=1))
    stage = ctx.enter_context(tc.tile_pool(name="stage", bufs=2))
    dram = ctx.enter_context(tc.tile_pool(name="dram", bufs=2, space="DRAM"))
    accp = ctx.enter_context(tc.tile_pool(name="accp", bufs=2, space="PSUM"))

    # ---- persistent SBUF tensors ----
    xT = sb.tile([128, 8 * 1024], BF)      # [d-part, dt*1024 + l]
    wq = sb.tile([128, 8 * 1024], BF)      # [d-part, dt*1024 + e]
    wk = sb.tile([128, 8 * 1024], BF)
    wv = sb.tile([128, 8 * 1024], BF)
    wo = sb.tile([128, 8 * 1024], BF)      # [e-part, et*1024 + e']
    qT = sb.tile([128, 8 * 1024], BF)      # [e-part, et*1024 + l]
    kT = sb.tile([128, 8 * 1024], BF)
    vb = sb.tile([128, 8 * 1040], BF)      # [l-part, lt*1040 + h*65 + eh]; col h*65+64 = mask
    oT = sb.tile([128, 8 * 1024], BF)      # [e-part, et*1024 + l]
    xtails = sb.tile([128, 64], BF)        # [d-part, dt*8 + bb]
    sel = sb.tile([64, 8], BF)             # one-hot row selector (per-core data)
    tri = sb.tile([128, 128], F32)         # tri[p,f] = 0 if p<=f else NEG
    kpm = sb.tile([128, 8], F32)           # key-padding 0/1 per [key-in-block, lt]
    ktail8 = sb.tile([8, 1], F32)          # key-padding for tail keys 1016..1023
    nsb = sb.tile([8, 3072], BF)           # my position's q|k|v for all batches
    fullg = sb.tile([64, 3072], BF)        # gathered: row n*8+bb

    # ---- ACT table warm-up: first ACTIVATE in the scalar stream is an Exp,
    # so the exp_and_others table set loads once, early.
    scr = stage.tile([1, 8], F32, name="scr")
    scr2 = stage.tile([1, 8], F32, name="scr2")
    nc.vector.memset(scr[:], 0.0)
    nc.scalar.activation(scr2[:], scr[:], EXP, scale=1.0)

    # ---- input DMAs, spread across engine queues ----
    # scalar queue: the four big weight matrices (order = need order)
    for dt in range(8):
        r = slice(dt * 128, dt * 128 + 128)
        nc.scalar.dma_start(wv[:, bass.ts(dt, 1024)], aps["wvT"][r, :])
    for dt in range(8):
        r = slice(dt * 128, dt * 128 + 128)
        nc.scalar.dma_start(wq[:, bass.ts(dt, 1024)], aps["wqT"][r, :])
    for dt in range(8):
        r = slice(dt * 128, dt * 128 + 128)
        nc.scalar.dma_start(wk[:, bass.ts(dt, 1024)], aps["wkT"][r, :])
    for dt in range(8):
        r = slice(dt * 128, dt * 128 + 128)
        nc.scalar.dma_start(wo[:, bass.ts(dt, 1024)], aps["woutT"][r, :])
    # sync queue: x + small tensors (xtails first: ns matmuls need them)
    for dt in range(8):
        r = slice(dt * 128, dt * 128 + 128)
        nc.sync.dma_start(xtails[:, bass.ts(dt, 8)], aps["xtails"][r, :])
    nc.sync.dma_start(sel[:], aps["sel"][:])
    nc.sync.dma_start(tri[:], aps["tri"][:])
    nc.sync.dma_start(kpm[:], aps["kpm"][:])
    nc.sync.dma_start(ktail8[:], aps["ktail8"][:])
    for dt in range(8):
        r = slice(dt * 128, dt * 128 + 128)
        nc.sync.dma_start(xT[:, bass.ts(dt, 1024)], aps["xT"][r, :])

    # ---- phase 1: ns projections for my position (all batches), ASAP ----
    with tc.tile_pool(name="wnsp", bufs=4) as wnsp, \
         tc.tile_pool(name="nsps", bufs=1, space="PSUM") as nsp_pool:
        pps = [nsp_pool.tile([8, 512], F32, name=f"pp{ck}") for ck in range(6)]
        for dt in range(8):
            wt = wnsp.tile([128, 3072], BF, name="wt")
            nc.gpsimd.dma_start(wt[:], aps["wnsT"][dt * 128: dt * 128 + 128, :])
            for ck in range(6):
                nc.tensor.matmul(
                    pps[ck][:],
                    xtails[:, bass.ts(dt, 8)],
                    wt[:, bass.ts(ck, 512)],
                    start=(dt == 0),
                    stop=(dt == 7),
                )
        for ck in range(6):
            nc.vector.tensor_copy(nsb[:, bass.ts(ck, 512)], pps[ck][:])

    gin = dram.tile([8, 3072], BF, name="gin")
    gout = dram.tile([64, 3072], BF, name="gout")
    nc.gpsimd.dma_start(gin[:], nsb[:])
    nc.gpsimd.collective_compute(
        "AllGather",
        mybir.AluOpType.bypass,
        replica_groups=[list(range(NCORES))],
        ins=[gin.opt()],
        outs=[gout.opt()],
    )
    nc.gpsimd.dma_start(fullg[:], gout[:])

    # ---- helpers ----
    # V projection block: natural [l, e] layout with per-head mask column;
    # key-padding folded in via per-partition scalar multiply during the
    # PSUM->SBUF copy.  g0 attention needs only lt 0..3, so lt 4..7 are
    # emitted later as filler while waiting for the AllGather.
    def v_block(lt):
        m = 120 if lt == 7 else 128          # tail rows 1016.. come from ns
        for eg in range(2):
            acc = accp.tile([128, 512], F32, name="acc")
            for dt in range(8):
                nc.tensor.matmul(
                    acc[0:m, :],
                    xT[:, dt * 1024 + lt * 128: dt * 1024 + lt * 128 + m],
                    wv[:, dt * 1024 + eg * 512: dt * 1024 + eg * 512 + 512],
                    start=(dt == 0),
                    stop=(dt == 7),
                )
            dstv = vb[0:m, lt * 1040 + eg * 520: lt * 1040 + eg * 520 + 520].rearrange(
                "p (h x) -> p h x", h=8, x=65
            )[:, :, 0:64]
            srcv = acc[0:m, :].rearrange("p (h x) -> p h x", h=8, x=64)
            nc.vector.tensor_scalar_mul(dstv, srcv, kpm[0:m, lt: lt + 1])
        # per-head mask column (= key-padding value); DVE write so it is
        # engine-serialized with the value casts (no DMA word-RMW races)
        dstm = vb[0:m, lt * 1040: lt * 1040 + 1040].rearrange(
            "p (h x) -> p h x", h=16, x=65
        )[:, :, 64:65]
        nc.vector.tensor_copy(
            dstm, kpm[0:m, lt: lt + 1].unsqueeze(1).to_broadcast((m, 16, 1))
        )

    def qk_round(et, lg):
        for (w, dst) in ((wq, qT), (wk, kT)):
            w0 = lg * 512
            w1 = 512 if lg == 0 else 504   # tail cols come from ns
            acc = accp.tile([128, 512], F32, name="acc")
            for dt in range(8):
                nc.tensor.matmul(
                    acc[:, 0:w1],
                    w[:, dt * 1024 + et * 128: dt * 1024 + et * 128 + 128],
                    xT[:, dt * 1024 + w0: dt * 1024 + w0 + w1],
                    start=(dt == 0),
                    stop=(dt == 7),
                )
            nc.any.tensor_copy(dst[:, et * 1024 + w0: et * 1024 + w0 + w1],
                               acc[:, 0:w1])

    def attn_pair_g(r, g, scp, ptp, opp):
        # Both heads of the pair in lockstep: their score matmuls contract
        # over disjoint partition halves (r0=0 vs 64 -> row groups 0-1 vs
        # 2-3 via auto tile_position), so back-to-back issue runs them
        # concurrently on the PE array.
        et = r
        hs = (2 * r, 2 * r + 1)
        nj = 4 if g == 0 else 8
        qbase = et * 1024 + g * 512
        pts = {h: ptp.tile([128, nj * 512], BF, name="pt") for h in hs}
        for jp in range(0, nj, 2):
            scs = {h: scp.tile([128, 1024], F32, name="sc") for h in hs}
            for s in range(2):
                j = jp + s
                lead = _lead(g, j)
                for h in hs:
                    r0 = (h % 2) * 64
                    nc.tensor.matmul(
                        scs[h][:, s * 512 + lead: s * 512 + 512],
                        kT[r0:r0 + 64,
                           et * 1024 + j * 128: et * 1024 + j * 128 + 128],
                        qT[r0:r0 + 64, qbase + lead: qbase + 512],
                        start=True,
                        stop=True,
                    )
            for h in hs:
                sc, pt = scs[h], pts[h]
                if g == 1 and jp < 4:
                    # both blocks fully valid: one fused exp over 1024 cols
                    nc.scalar.activation(
                        pt[:, jp * 512: jp * 512 + 1024], sc[:, 0:1024], EXP,
                        scale=0.125,
                    )
                else:
                    for s in range(2):
                        j = jp + s
                        lead = _lead(g, j)
                        nc.vector.tensor_tensor(
                            sc[:, s * 512 + lead: s * 512 + lead + 128],
                            sc[:, s * 512 + lead: s * 512 + lead + 128],
                            tri[:],
                            ADD,
                        )
                        nc.scalar.activation(
                            pt[:, j * 512 + lead: j * 512 + 512],
                            sc[:, s * 512 + lead: s * 512 + 512],
                            EXP,
                            scale=0.125,
                        )
        for h in hs:
            r0 = (h % 2) * 64
            pt = pts[h]
            op = opp.tile([128, 512], F32, name="op")
            for j in range(nj):
                lead = _lead(g, j)
                nc.tensor.matmul(
                    op[0:65, lead:512],
                    vb[:, j * 1040 + h * 65: j * 1040 + h * 65 + 65],
                    pt[:, j * 512 + lead: j * 512 + 512],
                    start=(j == 0),
                    stop=(j == nj - 1),
                )
            # normalize: oT = op * (1/den), den = op row 64 (ones-column).
            # custom-DVE ops ignore the input base partition -> stage the
            # denominator row to partition 0 first with a standard copy.
            den0 = stage.tile([1, 512], F32, name="den0")
            nc.vector.tensor_copy(den0[:], op[64:65, 0:512])
            rec32 = stage.tile([1, 512], F32, name="rec32")
            nc.vector.reciprocal_approx_fast(rec32[:], den0[:])
            bc = stage.tile([64, 512], F32, name="bc")
            nc.gpsimd.partition_broadcast(bc[:], rec32[:], channels=64)
            nc.vector.tensor_tensor(
                oT[r0:r0 + 64, qbase: qbase + 512],
                op[0:64, 0:512],
                bc[:],
                MULT,
            )

    def tails():
        # q/k tails, transposed via selection matmul
        for c2 in range(16):
            tp = accp.tile([128, 512], F32, name="acc")
            nc.tensor.matmul(
                tp[:, 0:8],
                fullg[:, c2 * 128: c2 * 128 + 128],
                sel[:],
                start=True,
                stop=True,
            )
            dst = qT if c2 < 8 else kT
            et2 = c2 % 8
            nc.any.tensor_copy(dst[:, et2 * 1024 + 1016: et2 * 1024 + 1024],
                               tp[:, 0:8])
        # v tails (natural layout), masked by tail key padding.  Build the
        # full 8x520 block (values + mask columns) in SBUF, then one aligned
        # contiguous DMA per half so no partial-word RMW races exist.
        for vg in range(2):
            tp = accp.tile([128, 512], F32, name="acc")
            nc.tensor.matmul(
                tp[0:8, :],
                sel[:],
                fullg[:, 2048 + vg * 512: 2048 + vg * 512 + 512],
                start=True,
                stop=True,
            )
            vt = stage.tile([8, 520], BF, name="vt")
            dvt = vt[:].rearrange("p (h x) -> p h x", h=8, x=65)
            nc.vector.tensor_scalar_mul(
                dvt[:, :, 0:64],
                tp[0:8, :].rearrange("p (h x) -> p h x", h=8, x=64),
                ktail8[:],
            )
            nc.vector.tensor_copy(
                dvt[:, :, 64:65],
                ktail8[:].unsqueeze(1).to_broadcast((8, 8, 1)),
            )
            nc.gpsimd.dma_start(
                vb[120:128, 7 * 1040 + vg * 520: 7 * 1040 + vg * 520 + 520],
                vt[:],
            )

    # ---- phases 2b-4: QK projections + attention, interleaved ----
    with tc.tile_pool(name="ptp", bufs=2) as ptp, \
         tc.tile_pool(name="scp", bufs=2, space="PSUM") as scp, \
         tc.tile_pool(name="opp", bufs=2, space="PSUM") as opp:

        def attn_round(r, gs):
            for g in gs:
                attn_pair_g(r, g, scp, ptp, opp)

        # g0 sweep first: no dependence on the (slow, ~100us) AllGather.
        # Only the work g0 needs runs before it (V lt0-3, QK cols 0:512);
        # V lt4-7 + QK cols 512:1016 fill the wait for the collective.
        for lt in range(4):
            v_block(lt)
        for r in range(8):
            qk_round(r, 0)
            attn_round(r, (0,))
        for lt in range(4, 8):
            v_block(lt)
        for r in range(8):
            qk_round(r, 1)
        tails()
        for r in range(4):
            attn_round(r, (1,))

        # ---- phase 5: output projection split by contraction half ----
        # et 0..3 (heads 0..7, complete after g1 round 3) interleave with the
        # ACT-bound g1 rounds 4..7 as PE filler; et 4..7 + merge at the end.
        with tc.tile_pool(name="pop", bufs=1) as pop:
            po = pop.tile([128, 16 * 512], BF)   # bf16 partials per (lt, eg)

            def outproj_half1(lts):
                for lt in lts:
                    for eg in range(2):
                        acc = accp.tile([128, 512], F32, name="acc")
                        for et in range(4):
                            nc.tensor.matmul(
                                acc[:],
                                oT[:, et * 1024 + lt * 128:
                                   et * 1024 + lt * 128 + 128],
                                wo[:, et * 1024 + eg * 512:
                                   et * 1024 + eg * 512 + 512],
                                start=(et == 0),
                                stop=(et == 3),
                            )
                        nc.vector.tensor_copy(
                            po[:, (lt * 2 + eg) * 512: (lt * 2 + eg) * 512 + 512],
                            acc[:],
                        )

            outproj_half1((0, 1))
            attn_round(4, (1,))
            outproj_half1((2, 3))
            attn_round(5, (1,))
            outproj_half1((4, 5))
            attn_round(6, (1,))
            outproj_half1((6, 7))
            attn_round(7, (1,))

            for lt in range(8):
                for eg in range(2):
                    acc = accp.tile([128, 512], F32, name="acc")
                    for et in range(4, 8):
                        nc.tensor.matmul(
                            acc[:],
                            oT[:, et * 1024 + lt * 128:
                               et * 1024 + lt * 128 + 128],
                            wo[:, et * 1024 + eg * 512:
                               et * 1024 + eg * 512 + 512],
                            start=(et == 4),
                            stop=(et == 7),
                        )
                    ys = stage.tile([128, 512], BF, name="ys")
                    nc.vector.tensor_tensor(
                        ys[:],
                        acc[:],
                        po[:, (lt * 2 + eg) * 512: (lt * 2 + eg) * 512 + 512],
                        ADD,
                    )
                    nc.gpsimd.dma_start(
                        aps["y"][lt * 128: lt * 128 + 128,
                                 eg * 512: eg * 512 + 512],
                        ys[:],
                    )

    if DEBUG:
        for nm, t in (("dqT", qT), ("dkT", kT), ("dvb", vb), ("doT", oT),
                      ("dfullg", fullg), ("dnsb", nsb)):
            nc.gpsimd.dma_start(aps[nm][:, :], t[:])


def _build():
    if "nc" in _CACHE:
        return _CACHE["nc"]
    nc = bacc.Bacc("TRN2", target_bir_lowering=False, debug=False, num_devices=NCORES)
    aps = {}
    for name, shape, dt in [
        ("xT", [1024, 1024], BF),
        ("wqT", [1024, 1024], BF),
        ("wkT", [1024, 1024], BF),
        ("wvT", [1024, 1024], BF),
        ("woutT", [1024, 1024], BF),
        ("wnsT", [1024, 3072], BF),
        ("xtails", [1024, 8], BF),
        ("sel", [64, 8], BF),
        ("tri", [128, 128], F32),
        ("kpm", [128, 8], F32),
        ("ktail8", [8, 1], F32),
    ]:
        aps[name] = nc.dram_tensor(name, shape, dt, kind="ExternalInput").ap()
    aps["y"] = nc.dram_tensor("y", [1024, 1024], BF, kind="ExternalOutput").ap()
    if DEBUG:
        for nm, shape in (("dqT", [128, 8192]), ("dkT", [128, 8192]),
                          ("dvb", [128, 8320]), ("doT", [128, 8192]),
                          ("dfullg", [64, 3072]), ("dnsb", [8, 3072])):
            aps[nm] = nc.dram_tensor(nm, shape, BF, kind="ExternalOutput").ap()

    with tile.TileContext(nc) as tc:
        _attention_kernel(tc, aps)
    nc.compile()
    _CACHE["nc"] = nc
    return nc


def kernel(x, key_padding_mask, Wq_s, Wk_s, Wv_s, Wq_ns, Wk_ns, Wv_ns, W_out, **kw):
    x = np.asarray(x, np.float32)
    mask = np.asarray(key_padding_mask)
    bf = ml_dtypes.bfloat16

    wqT = np.ascontiguousarray(np.asarray(Wq_s, np.float32).T.astype(bf))
    wkT = np.ascontiguousarray(np.asarray(Wk_s, np.float32).T.astype(bf))
    wvT = np.ascontiguousarray(np.asarray(Wv_s, np.float32).T.astype(bf))
    woT = np.ascontiguousarray(np.asarray(W_out, np.float32).T.astype(bf))
    tri = np.where(
        np.arange(128)[:, None] <= np.arange(128)[None, :], 0.0, NEG
    ).astype(np.float32)

    Wq_ns = np.asarray(Wq_ns, np.float32)
    Wk_ns = np.asarray(Wk_ns, np.float32)
    Wv_ns = np.asarray(Wv_ns, np.float32)

    in_maps = []
    for c in range(NCORES):
        xT = np.ascontiguousarray(x[c].T.astype(bf))
        xtails = np.ascontiguousarray(x[:, LS + c, :].T.astype(bf))  # [1024 d, 8 bb]
        wnsT = np.ascontiguousarray(
            np.concatenate([Wq_ns[c].T, Wk_ns[c].T, Wv_ns[c].T], axis=1).astype(bf)
        )
        selm = np.zeros((64, 8), bf)
        for n in range(NS):
            selm[n * 8 + c, n] = 1.0
        mk = mask[c].astype(np.float32)                 # [1024] 0/1
        kpm = np.ascontiguousarray(mk.reshape(8, 128).T)   # [128 key, lt]
        ktail8 = np.ascontiguousarray(mk[LS:].reshape(8, 1))
        in_maps.append(
            {
                "xT": xT,
                "wqT": wqT,
                "wkT": wkT,
                "wvT": wvT,
                "woutT": woT,
                "wnsT": wnsT,
                "xtails": xtails,
                "sel": selm,
                "tri": tri,
                "kpm": kpm,
                "ktail8": ktail8,
            }
        )

    nc = _build()
    res = run_bass_kernel_spmd(nc, in_maps, list(range(NCORES)), trace=TRACE)
    _CACHE["exec_time_ns"] = res.exec_time_ns
    _CACHE["res"] = res
    out = np.stack([np.asarray(res.results[c]["y"]) for c in range(NCORES)], axis=0)
    return out.astype(np.float32)

